# revision 48
# baseline (speedup 1.0000x reference)
# kernel.py — fused causal ReLU-attention (qkv proj + q@k^T + relu/causal + @v)
# for Trainium2, 8 NeuronCores, batch-parallel (1 batch element per core).
#
# Self-contained: hardcodes shapes B,T,C = 8,1024,768, nh=12, hs=64.
# Host-side prep: x is uploaded pre-transposed; W is uploaded pre-blocked
# (per 128-col slice, k-tiles packed) so every DMA is full-bandwidth.
import os
import sys

for p in ("/opt/trn_rl_repo", "/root/.axon_site", "/root/.axon_site/_ro/trn_rl_repo"):
    if os.path.isdir(p) and p not in sys.path:
        sys.path.append(p)

import numpy as np

import concourse.bass as bass
import concourse.mybir as mybir
import concourse.tile as tile
from concourse import bacc
from concourse import bass_utils
from concourse.masks import make_identity

F32 = mybir.dt.float32
BF16 = mybir.dt.bfloat16
AF = mybir.ActivationFunctionType
ALU = mybir.AluOpType

B, T, C = 8, 1024, 768
NH, HS = 12, 64
SCALE = 1.0 / 8.0  # 1/sqrt(64)
P = 128
NT = T // P    # 8 t-tiles
KC = C // P    # 6 c-tiles (contraction)
NPAIR = NH // 2  # 6 head pairs
TCH = 512      # t1 chunk size
NCH = T // TCH  # 2 chunks


def build_nc(n_cores=8):
    nc = bacc.Bacc("TRN2", target_bir_lowering=False, debug=False,
                   num_devices=n_cores)

    xt_d = nc.dram_tensor("xt", [C, T], BF16, kind="ExternalInput").ap()
    wqk_d = nc.dram_tensor("wqk", [12 * P, C], BF16,
                           kind="ExternalInput").ap()
    wv_d = nc.dram_tensor("wv", [C, C], BF16, kind="ExternalInput").ap()
    b_d = nc.dram_tensor("b", [3 * C], F32, kind="ExternalInput").ap()
    y_d = nc.dram_tensor("y", [T, C], BF16, kind="ExternalOutput").ap()

    with tile.TileContext(nc) as tc:
        _emit(nc, tc, xt_d, wqk_d, wv_d, b_d, y_d)

    nc.compile()
    return nc


def _emit(nc, tc, xt_d, wqk_d, wv_d, b_d, y_d):
    from contextlib import ExitStack

    with ExitStack() as ctx:
        pp = ctx.enter_context(tc.tile_pool(name="persist", bufs=1))
        ybp = ctx.enter_context(tc.tile_pool(name="yblk", bufs=4))
        ap_pool = ctx.enter_context(tc.tile_pool(name="attp", bufs=30))
        qps = ctx.enter_context(
            tc.tile_pool(name="qkv_psum", bufs=2, space="PSUM"))
        a_ps = ctx.enter_context(
            tc.tile_pool(name="att_psum", bufs=5, space="PSUM"))
        y_ps = ctx.enter_context(
            tc.tile_pool(name="y_psum", bufs=1, space="PSUM"))

        # ---- constants ----
        garbage = pp.tile([P, TCH], BF16, tag="garb", name="garb")
        nc.vector.memset(garbage[:], 0.0)

        def dummy_mm():
            # PE p-state warmup / stall filler: no DMA deps, result unused.
            # Uses y_psum (idle until the attention phase) so fillers never
            # block on an open accumulation chain's buffer.
            scr = y_ps.tile([P, TCH], F32, tag="yps", name="yps")
            nc.tensor.matmul(scr[:], garbage[:, 0:P], garbage[:],
                             start=True, stop=True)

        for _ in range(6):
            dummy_mm()

        bqk = pp.tile([P, 12], F32, tag="bqk", name="bqk")
        nc.scalar.dma_start(bqk[:], b_d[0:2 * C].rearrange("(a p) -> p a", p=P))
        bv_row = pp.tile([1, C], F32, tag="bvrow", name="bvrow")
        nc.scalar.dma_start(bv_row[:],
                          b_d[2 * C:3 * C].rearrange("(o a) -> o a", o=1))
        # bf16 copies for the K=1 bias matmul in the v projection
        bv8 = pp.tile([1, C], BF16, tag="bv8", name="bv8")
        nc.vector.tensor_copy(bv8[:], bv_row[:])
        ones1 = pp.tile([1, P], BF16, tag="ones1", name="ones1")
        nc.gpsimd.memset(ones1[:], 1.0)

        # master relu/causal mask, pre-scaled by SCALE:
        #   cols [0,384) = 0 ; [384,512) = (col-384>=part ? SCALE : 0) ;
        #   [512,896) = SCALE
        mstr = pp.tile([P, 896], F32, tag="mstr", name="mstr")
        nc.gpsimd.memset(mstr[:, 0:384], 0.0)
        nc.gpsimd.memset(mstr[:, 384:896], SCALE)
        nc.gpsimd.affine_select(
            out=mstr[:, 384:512], in_=mstr[:, 384:512],
            compare_op=ALU.is_ge, fill=0.0, base=0,
            pattern=[[1, P]], channel_multiplier=-1)

        # 0/1 fix mask: cols [0,128)=0 ; [128,256) = (col-128>=part ? 1 : 0)
        mstr2 = pp.tile([P, 256], BF16, tag="mstr2", name="mstr2")
        nc.gpsimd.memset(mstr2[:, 0:128], 0.0)
        nc.gpsimd.memset(mstr2[:, 128:256], 1.0)
        nc.gpsimd.affine_select(
            out=mstr2[:, 128:256], in_=mstr2[:, 128:256],
            compare_op=ALU.is_ge, fill=0.0, base=0,
            pattern=[[1, P]], channel_multiplier=-1)

        # ---- persistent activations (bf16) ----
        qkT = [pp.tile([P, T], BF16, tag=f"qkT{m}", name=f"qkT{m}")
               for m in range(12)]
        v_sb = [pp.tile([P, C], BF16, tag=f"v{i}", name=f"v{i}")
                for i in range(NT)]
        xT = pp.tile([P, KC * T], BF16, tag="xTall", name="xTall")

        def xt(k):
            return xT[:, k * T:(k + 1) * T]

        wq = [pp.tile([P, C], BF16, tag=f"wq{j}", name=f"wq{j}")
              for j in range(NPAIR)]
        wk = [pp.tile([P, C], BF16, tag=f"wk{j}", name=f"wk{j}")
              for j in range(NPAIR)]
        wv = [pp.tile([P, C], BF16, tag=f"wv{k}", name=f"wv{k}")
              for k in range(KC)]

        # ---- DMA schedule (2 HWDGE queues; emission order = queue order) --
        # interleave xt-t0 chunks with wv so the v-proj k-sweeps unlock
        # early; wq0/wk0 before xt-t1 so qk groups can fill the gap
        for k in range(KC):
            eng = nc.sync if k % 2 == 0 else nc.scalar
            eng.dma_start(xT[:, k * T:k * T + TCH],
                          xt_d[P * k:P * (k + 1), 0:TCH])
            eng.dma_start(wv[k][:], wv_d[P * k:P * (k + 1), :])
        nc.scalar.dma_start(wq[0][:], wqk_d[0:P, :])
        nc.sync.dma_start(wk[0][:], wqk_d[6 * P:7 * P, :])
        for k in range(KC):
            eng = nc.sync if k % 2 == 0 else nc.scalar
            eng.dma_start(xT[:, k * T + TCH:(k + 1) * T],
                          xt_d[P * k:P * (k + 1), TCH:T])
        # W q/k slices, consumption order (pair j uses wq[j], wk[j])
        for j in range(1, NPAIR):
            nc.scalar.dma_start(wq[j][:], wqk_d[P * j:P * (j + 1), :])
            nc.sync.dma_start(wk[j][:],
                              wqk_d[P * (6 + j):P * (7 + j), :])

        # ===== phase A: v projection, k-outer over 4-tile groups =========
        # 4 open psum chains; bias folded in as K=1 matmul
        def vproj_sweep(tg, n0, n1, fill):
            i0 = 4 * tg
            chains = [a_ps.tile([P, TCH], F32, tag="aps", name="aps")
                      for _ in range(4)]
            for k in range(KC):
                for ci in range(4):
                    i = i0 + ci
                    nc.tensor.matmul(
                        chains[ci][:, 0:n1 - n0],
                        xt(k)[:, P * i:P * (i + 1)],
                        wv[k][:, n0:n1],
                        start=(k == 0), stop=False)
                pass
            for ci in range(4):
                nc.tensor.matmul(
                    chains[ci][:, 0:n1 - n0], ones1[:], bv8[:, n0:n1],
                    start=False, stop=True)
            for ci in range(4):
                i = i0 + ci
                if ci % 2 == 0:
                    nc.vector.tensor_copy(v_sb[i][:, n0:n1],
                                          chains[ci][:, 0:n1 - n0])
                else:
                    nc.scalar.activation(v_sb[i][:, n0:n1],
                                         chains[ci][:, 0:n1 - n0],
                                         AF.Copy)

        gctr = 0

        def emit_qk_group(m, t):
            # m in [0,12): m<6 -> q block j=m (wq), else k block j=m-6 (wk)
            nonlocal gctr
            w_t = wq[m] if m < 6 else wk[m - 6]
            ps = qps.tile([P, TCH], F32, tag="qkvps", name="qkvps")
            for k in range(KC):
                nc.tensor.matmul(
                    ps[:],
                    w_t[:, P * k:P * (k + 1)],
                    xt(k)[:, TCH * t:TCH * (t + 1)],
                    start=(k == 0), stop=(k == KC - 1))
            gctr += 1
            if gctr % 2 == 0:
                nc.scalar.activation(qkT[m][:, TCH * t:TCH * (t + 1)],
                                     ps[:], AF.Identity,
                                     bias=bqk[:, m:m + 1])
            else:
                nc.vector.tensor_scalar(qkT[m][:, TCH * t:TCH * (t + 1)],
                                        ps[:], bqk[:, m:m + 1], None,
                                        ALU.add)

        def att_piece_params(c):
            c_lo = TCH * c
            out = []
            for r in range((TCH * (c + 1)) // P):
                t2_0 = P * r
                off = max(0, t2_0 - c_lo)   # no widening: z is always 0
                out.append((r, off, 0, TCH - off, t2_0 >= c_lo))
            return out

        # v-proj t0 sweeps (fill k-step supply stalls with dummies), then the
        # first qk groups (wq0/wk0 arrive before xt-t1), then t1 sweeps
        vproj_sweep(0, 0, 512, fill=True)
        vproj_sweep(0, 512, 768, fill=False)
        emit_qk_group(0, 0)
        emit_qk_group(6, 0)
        vproj_sweep(1, 0, 512, fill=False)
        vproj_sweep(1, 512, 768, fill=False)
        emit_qk_group(0, 1)
        emit_qk_group(6, 1)

        # ======= attention: per pair; att(j) interleaved with qk(j+1) =======
        # av runs in natural layout: out yn[t1-block 128, head-pair 128]
        # with the stored att piece [t2, t1] as the stationary operand —
        # full PE width, y written directly (no transposes).
        for j in range(NPAIR):
            qt, kt = qkT[j], qkT[6 + j]

            qk_sched = {}
            if j + 1 < NPAIR:
                qk_sched[2] = (j + 1, 0)
                qk_sched[4] = (j + 1, 1)
                qk_sched[6] = (7 + j, 0)
                qk_sched[8] = (7 + j, 1)
            att_work = [(c, p) for c in range(NCH)
                        for p in att_piece_params(c)]
            chunk_pieces = {0: [], 1: []}
            late_av = {0: [], 1: []}
            yp = {}
            dr = 0  # drain rotation counter

            def av_step(c, r2, hh, q, at2, off):
                # yn chain for (t1-block q, head hh) of chunk c, step r2.
                # out columns laid out so the drain feeds the (q, hd) DMA.
                # PSUM start-zeroing covers the written partitions across the
                # whole bank, so ONLY the chunk's first step sets start=True;
                # every other chain accumulates onto that bank-wide zero.
                t1b = 4 * c + q
                lo = P * t1b - TCH * c - off
                nc.tensor.matmul(
                    yp[c][:, P * q + 64 * hh:P * q + 64 * hh + 64],
                    at2[:, lo:lo + P],
                    v_sb[r2][:, P * j + 64 * hh:P * j + 64 * hh + 64],
                    start=(r2 == 0 and q == 0 and hh == 0),
                    stop=(r2 == t1b), skip_group_check=True)

            def emit_av(c, idx):
                # all yn steps fed by piece idx of chunk c (just-in-time).
                # The diagonal step (q == r2 within chunk) of a Pool-fixed
                # piece is deferred: its wedge columns land late. It is a
                # stop-step (start=False) so ordering is free — except the
                # r2==0 piece, which must emit inline (start=True).
                if c not in yp:
                    yp[c] = y_ps.tile([P, TCH], F32, tag="yps", name="yps")
                (r2, hh, off, n2, at2, sc) = chunk_pieces[c][idx]
                for q in range(4):
                    t1b = 4 * c + q
                    if r2 > t1b:
                        continue
                    if sc is not None and r2 == t1b and r2 != 0:
                        late_av[c].append((r2, hh, q, at2, off))
                    else:
                        av_step(c, r2, hh, q, at2, off)

            for wi, (c, (r, offp, z, n, diag)) in enumerate(att_work):
                if wi in qk_sched:
                    emit_qk_group(*qk_sched[wi])
                c_lo = TCH * c
                for hh in range(2):
                    h0 = 64 * hh
                    ps = a_ps.tile([P, TCH], F32, tag="aps", name="aps")
                    nc.tensor.matmul(
                        ps[:, 0:n],
                        kt[h0:h0 + 64, P * r:P * r + P],
                        qt[h0:h0 + 64, c_lo + offp:TCH * (c + 1)],
                        start=True, stop=True,
                        tile_position=(h0, 0))
                    at = ap_pool.tile([P, TCH], BF16, tag="attp",
                                      name="attp")
                    dr += 1
                    sc = None
                    if diag:
                        # masked drain: alternate DVE full-mask vs
                        # ACT relu + Pool sbuf-side wedge fix. Last pieces
                        # of each chunk always take the lower-latency DVE
                        # path (their av cannot be deferred far).
                        last2 = r >= (TCH * (c + 1)) // P - 1
                        if last2 or dr % 2 == 0:
                            nc.vector.scalar_tensor_tensor(
                                at[:, 0:n], ps[:, 0:n], 0.0,
                                mstr[:, 384 - z:384 - z + n],
                                ALU.max, ALU.mult)
                        else:
                            nc.scalar.activation(at[:, 0:n], ps[:, 0:n],
                                                 AF.Relu, scale=SCALE)
                            nc.gpsimd.tensor_tensor(
                                at[:, 0:z + P], at[:, 0:z + P],
                                mstr2[:, P - z:2 * P - z + z], ALU.mult)
                            sc = z + P
                    else:
                        # plain relu drain: alternate ACT / DVE
                        if dr % 2 == 1:
                            nc.vector.tensor_scalar(
                                at[:, 0:n], ps[:, 0:n], SCALE, 0.0,
                                ALU.mult, ALU.max)
                        else:
                            nc.scalar.activation(at[:, 0:n], ps[:, 0:n],
                                                 AF.Relu, scale=SCALE)
                    chunk_pieces[c].append((r, hh, offp, n, at, sc))

                # just-in-time av: consume drains ~3 pieces behind
                pi = len(chunk_pieces[c]) // 2 - 1  # piece index within c
                if pi >= 3:
                    emit_av(c, 2 * (pi - 3))
                    emit_av(c, 2 * (pi - 3) + 1)

                if (c == 0 and r == 3) or (c == 1 and r == 7):
                    # flush remaining av pieces for chunk c
                    npc = len(chunk_pieces[c])
                    done = 2 * (npc // 2 - 3)
                    for idx in range(max(0, done), npc):
                        emit_av(c, idx)
                    for (r2, hh2, q2, at2, off2) in late_av[c]:
                        av_step(c, r2, hh2, q2, at2, off2)
                    late_av[c] = []
                    # drain yn psum -> yb4 on both engines, then one DMA
                    c_hi = TCH * (c + 1)
                    yb4 = ybp.tile([P, TCH], BF16, tag="yb", name="yb")
                    if c == 0:
                        nc.scalar.activation(yb4[:, 0:256],
                                             yp[c][:, 0:256], AF.Copy)
                        nc.vector.tensor_copy(yb4[:, 256:512],
                                              yp[c][:, 256:512])
                    else:
                        nc.vector.tensor_copy(yb4[:, 0:256],
                                              yp[c][:, 0:256])
                        nc.scalar.activation(yb4[:, 256:512],
                                             yp[c][:, 256:512], AF.Copy)
                    deng = nc.sync if (j + c) % 2 == 0 else nc.scalar
                    deng.dma_start(
                        y_d[c_lo:c_hi, P * j:P * (j + 1)].rearrange(
                            "(q p) c -> p q c", p=P),
                        yb4[:].rearrange("p (q c) -> p q c", q=4))


def _ensure_ntff_hook():
    """Register the axon NTFF profiling hook if the image's antenv lacks
    axon_hooks (bass_utils hard-imports it on the trace=True path)."""
    import types
    try:
        from antenv import axon_hooks  # noqa: F401
        return
    except ImportError:
        pass
    import antenv
    mod = types.ModuleType("antenv.axon_hooks")
    mod._hook = None

    def set_axon_ntff_profile_hook(h):
        mod._hook = h

    def get_axon_ntff_profile_hook():
        return mod._hook

    mod.set_axon_ntff_profile_hook = set_axon_ntff_profile_hook
    mod.get_axon_ntff_profile_hook = get_axon_ntff_profile_hook
    sys.modules["antenv.axon_hooks"] = mod
    antenv.axon_hooks = mod
    try:
        from trn_agent_boot.trn_boot import _ntff_profile_via_ctypes
        hook = _ntff_profile_via_ctypes("/opt/axon/libaxon_pjrt.so")
        if hook is not None:
            mod._hook = hook
    except Exception:
        pass


_NC_CACHE = None


def _get_nc():
    global _NC_CACHE
    if _NC_CACHE is None:
        _NC_CACHE = build_nc()
    return _NC_CACHE


def kernel(x, W_attn, b_attn, _trace=False):
    import ml_dtypes
    x = np.asarray(x, dtype=np.float32)
    w = np.asarray(W_attn).astype(ml_dtypes.bfloat16)
    b = np.ascontiguousarray(np.asarray(b_attn, dtype=np.float32))
    assert x.shape == (B, T, C) and w.shape == (C, 3 * C) and b.shape == (3 * C,)

    # host-side repack: x transposed per batch; W q/k m-sliced k-blocked
    xt = np.ascontiguousarray(
        np.transpose(x.astype(ml_dtypes.bfloat16), (0, 2, 1)))  # [B, C, T]
    wqk = np.ascontiguousarray(
        w[:, :2 * C].reshape(KC, P, 12, P).transpose(2, 1, 0, 3)
        .reshape(12 * P, C))                                     # [12P, C]
    wv = np.ascontiguousarray(w[:, 2 * C:])                      # [C, C]

    if _trace:
        _ensure_ntff_hook()
    nc = _get_nc()
    in_maps = [{"xt": xt[i], "wqk": wqk, "wv": wv, "b": b}
               for i in range(B)]
    res = bass_utils.run_bass_kernel_spmd(
        nc, in_maps, core_ids=list(range(B)), trace=_trace)
    y = np.stack([np.asarray(res.results[i]["y"]).astype(np.float32)
                  for i in range(B)], axis=0)
    if _trace:
        kernel.last_result = res
    return y
